# revision 12
# baseline (speedup 1.0000x reference)
"""Sliding-window (radius-8, K=17) single-head attention along W — v4.

Math (see host_consts): S[w',w] = g(w')·x(w) + alpha(w') (+terms that cancel
in softmax), g = M x + u with M = Wq^T Wk/sc; alpha rides the ACT exp bias;
the global constant c0 and fp16-range shift s0 are baked into the zero-pad
denominator term. Banded 136-col score/den/value matmuls in fp16, with an
additive -30000 band mask accumulated into the score PSUM by a PE matmul
(exp of masked entries underflows to 0, so den/value read exp output as-is).

Engine assignment per row of [C=128, W=256]:
  Sync   : input f/p chunk DMAs (HWDGE), x^T X-bar DMA-transposes, out DMAs
  Scalar : const DMAs (parallel queue), exp with alpha bias, alpha evict
  GpSimd : x = f + p (f32+f32 -> fp16)
  PE     : g (N=512/pair), alpha (N=1), mask+scores, den, value
  DVE    : g eviction (+u -> fp16), reciprocal of den, final normalize mul
"""

import numpy as np
from contextlib import ExitStack

import concourse.bacc as bacc
import concourse.mybir as mybir
import concourse.tile as tile
from concourse.bass_utils import run_bass_kernel_spmd

B, C, H, W = 2, 128, 64, 256
R = 8
NCORES = 8
ROWS = B * H // NCORES        # 16 (b, h) rows per core
CORES_PER_B = NCORES // B     # 4
F32 = mybir.dt.float32
F16 = mybir.dt.float16
EXP = mybir.ActivationFunctionType.Exp
MULT = mybir.AluOpType.mult
S0 = 7.0                      # constant score shift (softmax-invariant)
NB = 136                      # banded block width (128 + R)
CHUNK_ROWS = [2, 2, 4, 4, 4]  # input/x chunking (small first chunks)


def build_nc():
    nc = bacc.Bacc(trn_type="TRN2")
    f_ext = nc.dram_tensor("feature", [C, ROWS, W], F32, kind="ExternalInput")
    p_ext = nc.dram_tensor("position", [C, ROWS, W], F32, kind="ExternalInput")
    mt_ext = nc.dram_tensor("mt", [C, C], F16, kind="ExternalInput")
    ones_ext = nc.dram_tensor("ones", [C, C], F16, kind="ExternalInput")
    id_ext = nc.dram_tensor("ident", [C, C], F16, kind="ExternalInput")
    v_ext = nc.dram_tensor("vt", [C, 1], F16, kind="ExternalInput")
    u_ext = nc.dram_tensor("ut", [C, 1], F32, kind="ExternalInput")
    mask_ext = nc.dram_tensor("maskT", [C, 2 * NB], F16, kind="ExternalInput")
    oob_ext = nc.dram_tensor("oob16", [C, W], F16, kind="ExternalInput")
    out_ext = nc.dram_tensor("out", [C, ROWS, W], F16, kind="ExternalOutput")

    with tile.TileContext(nc) as tc, ExitStack() as ctx:
        const = ctx.enter_context(tc.tile_pool(name="const", bufs=1))
        xgp = ctx.enter_context(tc.tile_pool(name="xg", bufs=1))
        inp = ctx.enter_context(tc.tile_pool(name="inp", bufs=3))

        # input chunks on the Sync HWDGE queue, all issued up front
        x_sb = xgp.tile([C, ROWS * W], F16, tag="x")
        fts, pts = [], []
        r0s, szs = [], []
        rr0 = 0
        for nr in CHUNK_ROWS:
            r0s.append(rr0)
            szs.append(nr * W)
            ft = inp.tile([C, 4 * W], F32, tag="ft")
            nc.sync.dma_start(ft[:, : nr * W], f_ext[:, rr0 : rr0 + nr, :])
            pt = inp.tile([C, 4 * W], F32, tag="pt")
            nc.sync.dma_start(pt[:, : nr * W], p_ext[:, rr0 : rr0 + nr, :])
            fts.append(ft)
            pts.append(pt)
            rr0 += nr

        # consts on the Scalar HWDGE queue (parallel to the input loads)
        def cload(shape, dt, ext, tag):
            t = const.tile(shape, dt, tag=tag, name=tag)
            nc.scalar.dma_start(t[:], ext[:])
            return t

        mt_t = cload([C, C], F16, mt_ext, "mt")
        ones_t = cload([C, C], F16, ones_ext, "ones")
        ident = cload([C, C], F16, id_ext, "id")
        v_t = cload([C, 1], F16, v_ext, "v")
        u_t = cload([C, 1], F32, u_ext, "u")
        mask_t = cload([C, 2 * NB], F16, mask_ext, "mask")
        oob_t = cload([C, W], F16, oob_ext, "oob")

        # x = f + p as fp16 (gpsimd), split into 2-row ops so each pair's
        # x (and its transpose) is ready as early as possible
        for i, nr in enumerate(CHUNK_ROWS):
            for j in range(0, nr, 2):
                sl = slice((r0s[i] + j) * W, (r0s[i] + j + 2) * W)
                fsl = slice(j * W, (j + 2) * W)
                nc.gpsimd.tensor_add(x_sb[:, sl], fts[i][:, fsl], pts[i][:, fsl])

        # x^T per chunk via X-bar DMA transpose, issued as soon as each
        # x chunk exists: xt[p, j, c] = x[c, j*128 + p]
        xt_sb = xgp.tile([C, 2 * ROWS, 128], F16, tag="xt")
        for i, nr in enumerate(CHUNK_ROWS):
            sl = slice(r0s[i] * W, r0s[i] * W + szs[i])
            nc.sync.dma_start_transpose(
                xt_sb[:, 2 * r0s[i] : 2 * (r0s[i] + nr), :], x_sb[:, sl]
            )

        # touch Exp once so the ACT table loads during the input-DMA ramp
        warm = const.tile([C, 1], F32, tag="warm")
        nc.scalar.activation(warm[:], u_t[:], EXP)
        negs0 = const.tile([C, 1], F32, tag="negs0")
        nc.vector.memset(negs0[:], -S0)

        g_sb = xgp.tile([C, ROWS * W], F16, tag="g")

        attp = ctx.enter_context(tc.tile_pool(name="att", bufs=4))
        sbp = ctx.enter_context(tc.tile_pool(name="sb", bufs=2))
        ps_s = ctx.enter_context(tc.tile_pool(name="ps_s", bufs=3, space="PSUM"))
        ps_g = ctx.enter_context(tc.tile_pool(name="ps_g", bufs=2, space="PSUM"))
        ps_dn = ctx.enter_context(tc.tile_pool(name="ps_dn", bufs=2, space="PSUM"))
        ps_o = ctx.enter_context(tc.tile_pool(name="ps_o", bufs=1, space="PSUM"))

        # g = M x (+u at eviction): one N=512 matmul per row-pair, in two
        # 4-pair bursts (M^T stationary loaded once per burst)
        def g_block(r):
            xsl = slice(r * W, (r + 2) * W)
            g_ps = ps_g.tile([C, 2 * W], F32, tag="g")
            nc.tensor.matmul(g_ps[:], mt_t[:], x_sb[:, xsl], start=True, stop=True)
            nc.vector.tensor_scalar_add(g_sb[:, xsl], g_ps[:], u_t[:])

        # scores for one pair: alpha MMs, band mask, banded score MMs,
        # then exp with alpha bias (ACT) -> att tiles
        def scores_block(r):
            atts = []
            for rr in range(2):
                x_r = x_sb[:, (r + rr) * W : (r + rr + 1) * W]
                g_r = g_sb[:, (r + rr) * W : (r + rr + 1) * W]
                s_ps = ps_s.tile([C, 2 * NB + 2], F32, tag="s")
                nc.tensor.matmul(
                    s_ps[:, 2 * NB : 2 * NB + 1], x_r[:, 0:128], v_t[:],
                    start=True, stop=True,
                )
                nc.tensor.matmul(
                    s_ps[:, 2 * NB + 1 : 2 * NB + 2], x_r[:, 128:256], v_t[:],
                    start=True, stop=True,
                )
                nc.tensor.matmul(
                    s_ps[:, 0 : 2 * NB], ident[:], mask_t[:],
                    start=True, stop=False,
                )
                nc.tensor.matmul(
                    s_ps[:, 0:NB], g_r[:, 0:128], x_r[:, 0:NB],
                    start=False, stop=False,
                )
                nc.tensor.matmul(
                    s_ps[:, NB : 2 * NB], g_r[:, 128:256], x_r[:, W - NB : W],
                    start=False, stop=True,
                )
                al_sb = sbp.tile([C, 2], F32, tag="al")
                nc.scalar.add(al_sb[:], s_ps[:, 2 * NB : 2 * NB + 2], negs0[:])
                att = attp.tile([C, 2 * NB], F16, tag="att")
                nc.scalar.activation(
                    att[:, 0:NB], s_ps[:, 0:NB], EXP, bias=al_sb[:, 0:1]
                )
                nc.scalar.activation(
                    att[:, NB : 2 * NB], s_ps[:, NB : 2 * NB], EXP,
                    bias=al_sb[:, 1:2],
                )
                atts.append(att)
            return atts

        def denval_block(r, atts):
            den_ps = ps_dn.tile([C, 2 * W], F32, tag="dn")
            out_ps = ps_o.tile([C, 2 * W], F32, tag="out")
            for rr in range(2):
                att = atts[rr]
                o0 = rr * W
                nc.tensor.matmul(
                    den_ps[:, o0 : o0 + W], ones_t[:], oob_t[:],
                    start=True, stop=False,
                )
                nc.tensor.matmul(
                    den_ps[:, o0 : o0 + NB], ones_t[:], att[:, 0:NB],
                    start=False, stop=False,
                )
                nc.tensor.matmul(
                    den_ps[:, o0 + W - NB : o0 + W], ones_t[:], att[:, NB : 2 * NB],
                    start=False, stop=True,
                )
            for rr in range(2):
                att = atts[rr]
                o0 = rr * W
                xt0 = xt_sb[:, 2 * (r + rr), :]
                xt1 = xt_sb[:, 2 * (r + rr) + 1, :]
                nc.tensor.matmul(
                    out_ps[:, o0 : o0 + NB], xt0, att[:, 0:NB],
                    start=True, stop=True,
                )
                nc.tensor.matmul(
                    out_ps[:, o0 + 120 : o0 + NB], xt1, att[:, NB : NB + 16],
                    start=False, stop=True, skip_group_check=True,
                )
                nc.tensor.matmul(
                    out_ps[:, o0 + NB : o0 + W], xt1, att[:, NB + 16 : 2 * NB],
                    start=True, stop=True,
                )
            rden = sbp.tile([C, 2 * W], F32, tag="rd")
            nc.vector.reciprocal_approx_fast(out=rden[:], in_=den_ps[:])
            ostage = sbp.tile([C, 2 * W], F16, tag="ost")
            nc.vector.tensor_tensor(ostage[:], out_ps[:], rden[:], MULT)
            nc.sync.dma_start(out_ext[:, r : r + 2, :], ostage[:])

        # software-pipelined PE program: scores of pair p+1 are emitted
        # before den/value of pair p so PE never idles on exp(p)
        NP = ROWS // 2
        for pr in range(4):
            g_block(2 * pr)
        pend = scores_block(0)
        for p in range(NP):
            if p == 3:
                for pr in range(4):
                    g_block(8 + 2 * pr)
            nxt = scores_block(2 * (p + 1)) if p + 1 < NP else None
            denval_block(2 * p, pend)
            pend = nxt

    nc.compile()
    return nc


def host_consts(Wq, bq, Wk, bk):
    sc = np.float32(np.sqrt(np.float32(C)))
    Wq = Wq.astype(np.float64)
    Wk = Wk.astype(np.float64)
    bq = bq.astype(np.float64)
    bk = bk.astype(np.float64)
    M = (Wq.T @ Wk) / sc
    v = (Wk.T @ bq) / sc
    u = (Wq.T @ bk) / sc
    c0 = float(bq @ bk) / sc

    mt = np.ascontiguousarray(M.T).astype(np.float16)      # lhsT for g = M x
    vt = v.reshape(C, 1).astype(np.float16)
    ut = u.reshape(C, 1).astype(np.float32)
    ident = np.eye(C, dtype=np.float16)
    ones = np.ones((C, C), dtype=np.float16)

    # additive band mask in the banded block layout:
    # chunk0 cols 0..NB-1 (query w=col, key p),
    # chunk1 cols NB..2NB-1 (query w=120+(col-NB), key 128+p)
    mask = np.full((C, 2 * NB), -30000.0, dtype=np.float64)
    for pp in range(C):
        for col in range(NB):
            if abs(col - pp) <= R:
                mask[pp, col] = 0.0
            if abs((W - NB + col) - (128 + pp)) <= R:
                mask[pp, NB + col] = 0.0
    mask = mask.astype(np.float16)

    wgrid = np.arange(W)
    oob_row = np.maximum(0, R - wgrid) + np.maximum(0, wgrid - (W - 1 - R))
    oob16 = np.tile(
        (oob_row * np.exp(-c0 - S0) / C).astype(np.float16), (C, 1)
    )
    return mt, vt, ut, ident, ones, mask, oob16


def core_inputs(feature, position, Wq, bq, Wk, bk):
    mt, vt, ut, ident, ones, mask, oob16 = host_consts(Wq, bq, Wk, bk)
    in_maps = []
    for i in range(NCORES):
        b = i // CORES_PER_B
        h0 = (i % CORES_PER_B) * ROWS
        in_maps.append(
            {
                "feature": np.ascontiguousarray(
                    feature[b, :, h0 : h0 + ROWS, :], dtype=np.float32
                ),
                "position": np.ascontiguousarray(
                    position[b, :, h0 : h0 + ROWS, :], dtype=np.float32
                ),
                "mt": mt,
                "ones": ones,
                "ident": ident,
                "vt": vt,
                "ut": ut,
                "maskT": mask,
                "oob16": oob16,
            }
        )
    return in_maps


def kernel(feature, position, Wq, bq, Wk, bk):
    feature = np.asarray(feature, dtype=np.float32)
    position = np.asarray(position, dtype=np.float32)
    Wq = np.asarray(Wq, dtype=np.float32)
    bq = np.asarray(bq, dtype=np.float32)
    Wk = np.asarray(Wk, dtype=np.float32)
    bk = np.asarray(bk, dtype=np.float32)
    in_maps = core_inputs(feature, position, Wq, bq, Wk, bk)
    nc = build_nc()
    res = run_bass_kernel_spmd(nc, in_maps, list(range(NCORES)))
    out = np.empty((B, C, H, W), dtype=np.float32)
    for i in range(NCORES):
        b = i // CORES_PER_B
        h0 = (i % CORES_PER_B) * ROWS
        out[b, :, h0 : h0 + ROWS, :] = res.results[i]["out"].astype(np.float32)
    return out


# revision 16
# speedup vs baseline: 1.0569x; 1.0569x over previous
"""Sliding-window (radius-8, K=17) single-head attention along W — v4.

Math (see host_consts): S[w',w] = g(w')·x(w) + alpha(w') (+terms that cancel
in softmax), g = M x + u with M = Wq^T Wk/sc; alpha rides the ACT exp bias;
the global constant c0 and fp16-range shift s0 are baked into the zero-pad
denominator term. Banded 136-col score/den/value matmuls in fp16, with an
additive -30000 band mask accumulated into the score PSUM by a PE matmul
(exp of masked entries underflows to 0, so den/value read exp output as-is).

Engine assignment per row of [C=128, W=256]:
  Sync   : input f/p chunk DMAs (HWDGE), x^T X-bar DMA-transposes, out DMAs
  Scalar : const DMAs (parallel queue), exp with alpha bias, alpha evict
  GpSimd : x = f + p (f32+f32 -> fp16)
  PE     : g (N=512/pair), alpha (N=1), mask+scores, den, value
  DVE    : g eviction (+u -> fp16), reciprocal of den, final normalize mul
"""

import numpy as np
from contextlib import ExitStack

import concourse.bacc as bacc
import concourse.mybir as mybir
import concourse.tile as tile
from concourse.bass_utils import run_bass_kernel_spmd

B, C, H, W = 2, 128, 64, 256
R = 8
NCORES = 8
ROWS = B * H // NCORES        # 16 (b, h) rows per core
CORES_PER_B = NCORES // B     # 4
F32 = mybir.dt.float32
F16 = mybir.dt.float16
EXP = mybir.ActivationFunctionType.Exp
MULT = mybir.AluOpType.mult
S0 = 7.0                      # constant score shift (softmax-invariant)
NB = 136                      # banded block width (128 + R)
CHUNK_ROWS = [2, 2, 4, 4, 4]  # input/x chunking (small first chunks)


def build_nc():
    nc = bacc.Bacc(trn_type="TRN2")
    f_ext = nc.dram_tensor("feature", [C, ROWS, W], F32, kind="ExternalInput")
    p_ext = nc.dram_tensor("position", [C, ROWS, W], F32, kind="ExternalInput")
    mt_ext = nc.dram_tensor("mt", [C, C], F16, kind="ExternalInput")
    ones_ext = nc.dram_tensor("ones", [C, C], F16, kind="ExternalInput")
    id_ext = nc.dram_tensor("ident", [C, C], F16, kind="ExternalInput")
    v_ext = nc.dram_tensor("vt", [C, 1], F16, kind="ExternalInput")
    u_ext = nc.dram_tensor("ut", [C, 1], F32, kind="ExternalInput")
    mask_ext = nc.dram_tensor("maskT", [C, 2 * NB], F16, kind="ExternalInput")
    oob_ext = nc.dram_tensor("oob16", [C, W], F16, kind="ExternalInput")
    out_ext = nc.dram_tensor("out", [C, ROWS, W], F16, kind="ExternalOutput")

    with tile.TileContext(nc) as tc, ExitStack() as ctx:
        const = ctx.enter_context(tc.tile_pool(name="const", bufs=1))
        xgp = ctx.enter_context(tc.tile_pool(name="xg", bufs=1))
        inp = ctx.enter_context(tc.tile_pool(name="inp", bufs=5))

        # input chunks on the Sync HWDGE queue, all issued up front
        x_sb = xgp.tile([C, ROWS * W], F16, tag="x")
        fts, pts = [], []
        r0s, szs = [], []
        rr0 = 0
        for nr in CHUNK_ROWS:
            r0s.append(rr0)
            szs.append(nr * W)
            ft = inp.tile([C, 4 * W], F32, tag="ft")
            nc.sync.dma_start(ft[:, : nr * W], f_ext[:, rr0 : rr0 + nr, :])
            pt = inp.tile([C, 4 * W], F32, tag="pt")
            nc.sync.dma_start(pt[:, : nr * W], p_ext[:, rr0 : rr0 + nr, :])
            fts.append(ft)
            pts.append(pt)
            rr0 += nr

        # consts on the Scalar HWDGE queue (parallel to the input loads)
        def cload(shape, dt, ext, tag):
            t = const.tile(shape, dt, tag=tag, name=tag)
            nc.scalar.dma_start(t[:], ext[:])
            return t

        mt_t = cload([C, C], F16, mt_ext, "mt")
        ones_t = cload([C, C], F16, ones_ext, "ones")
        ident = cload([C, C], F16, id_ext, "id")
        v_t = cload([C, 1], F16, v_ext, "v")
        u_t = cload([C, 1], F32, u_ext, "u")
        mask_t = cload([C, 2 * NB], F16, mask_ext, "mask")
        oob_t = cload([C, W], F16, oob_ext, "oob")

        # x = f + p as fp16, split into 2-row ops so each pair's x (and its
        # transpose) is ready as early as possible; the first two adds run
        # on DVE (idle during the ramp), the rest on gpsimd
        nadd = 0
        for i, nr in enumerate(CHUNK_ROWS):
            for j in range(0, nr, 2):
                sl = slice((r0s[i] + j) * W, (r0s[i] + j + 2) * W)
                fsl = slice(j * W, (j + 2) * W)
                eng = nc.vector if nadd < 2 else nc.gpsimd
                eng.tensor_add(x_sb[:, sl], fts[i][:, fsl], pts[i][:, fsl])
                nadd += 1

        # x^T per chunk via X-bar DMA transpose, issued as soon as each
        # x chunk exists: xt[p, j, c] = x[c, j*128 + p]
        xt_sb = xgp.tile([C, 2 * ROWS, 128], F16, tag="xt")
        for i, nr in enumerate(CHUNK_ROWS):
            sl = slice(r0s[i] * W, r0s[i] * W + szs[i])
            nc.sync.dma_start_transpose(
                xt_sb[:, 2 * r0s[i] : 2 * (r0s[i] + nr), :], x_sb[:, sl]
            )

        # touch Exp once so the ACT table loads during the input-DMA ramp
        warm = const.tile([C, 1], F32, tag="warm")
        nc.scalar.activation(warm[:], u_t[:], EXP)
        negs0 = const.tile([C, 1], F32, tag="negs0")
        nc.vector.memset(negs0[:], -S0)

        g_sb = xgp.tile([C, ROWS * W], F16, tag="g")

        attp = ctx.enter_context(tc.tile_pool(name="att", bufs=4))
        sbp = ctx.enter_context(tc.tile_pool(name="sb", bufs=2))
        ps_s = ctx.enter_context(tc.tile_pool(name="ps_s", bufs=3, space="PSUM"))
        ps_g = ctx.enter_context(tc.tile_pool(name="ps_g", bufs=2, space="PSUM"))
        ps_dn = ctx.enter_context(tc.tile_pool(name="ps_dn", bufs=2, space="PSUM"))
        ps_o = ctx.enter_context(tc.tile_pool(name="ps_o", bufs=1, space="PSUM"))

        # g = M x (+u at eviction): one N=512 matmul per row-pair, in two
        # 4-pair bursts (M^T stationary loaded once per burst)
        def g_block(r):
            xsl = slice(r * W, (r + 2) * W)
            g_ps = ps_g.tile([C, 2 * W], F32, tag="g")
            nc.tensor.matmul(g_ps[:], mt_t[:], x_sb[:, xsl], start=True, stop=True)
            nc.vector.tensor_scalar_add(g_sb[:, xsl], g_ps[:], u_t[:])

        # scores for one pair: alpha MMs, band mask, banded score MMs,
        # then exp with alpha bias (ACT) -> att tiles
        def scores_block(r):
            s_tiles = []
            # alpha MMs first (x chunks as stationaries), both rows
            for rr in range(2):
                x_r = x_sb[:, (r + rr) * W : (r + rr + 1) * W]
                s_ps = ps_s.tile([C, 2 * NB + 2], F32, tag="s")
                s_tiles.append(s_ps)
                nc.tensor.matmul(
                    s_ps[:, 2 * NB : 2 * NB + 1], x_r[:, 0:128], v_t[:],
                    start=True, stop=True,
                )
                nc.tensor.matmul(
                    s_ps[:, 2 * NB + 1 : 2 * NB + 2], x_r[:, 128:256], v_t[:],
                    start=True, stop=True,
                )
            # band mask for both rows with ident loaded once
            for rr in range(2):
                nc.tensor.matmul(
                    s_tiles[rr][:, 0 : 2 * NB], ident[:], mask_t[:],
                    start=True, stop=False,
                )
            atts = []
            for rr in range(2):
                x_r = x_sb[:, (r + rr) * W : (r + rr + 1) * W]
                g_r = g_sb[:, (r + rr) * W : (r + rr + 1) * W]
                s_ps = s_tiles[rr]
                nc.tensor.matmul(
                    s_ps[:, 0:NB], g_r[:, 0:128], x_r[:, 0:NB],
                    start=False, stop=False,
                )
                nc.tensor.matmul(
                    s_ps[:, NB : 2 * NB], g_r[:, 128:256], x_r[:, W - NB : W],
                    start=False, stop=True,
                )
                al_sb = sbp.tile([C, 2], F32, tag="al")
                nc.scalar.add(al_sb[:], s_ps[:, 2 * NB : 2 * NB + 2], negs0[:])
                att = attp.tile([C, 2 * NB], F16, tag="att")
                nc.scalar.activation(
                    att[:, 0:NB], s_ps[:, 0:NB], EXP, bias=al_sb[:, 0:1]
                )
                nc.scalar.activation(
                    att[:, NB : 2 * NB], s_ps[:, NB : 2 * NB], EXP,
                    bias=al_sb[:, 1:2],
                )
                atts.append(att)
            return atts

        def denval_block(r, atts):
            den_ps = ps_dn.tile([C, 2 * W], F32, tag="dn")
            out_ps = ps_o.tile([C, 2 * W], F32, tag="out")
            for rr in range(2):
                att = atts[rr]
                o0 = rr * W
                nc.tensor.matmul(
                    den_ps[:, o0 : o0 + W], ones_t[:], oob_t[:],
                    start=True, stop=False,
                )
                nc.tensor.matmul(
                    den_ps[:, o0 : o0 + NB], ones_t[:], att[:, 0:NB],
                    start=False, stop=False,
                )
                nc.tensor.matmul(
                    den_ps[:, o0 + W - NB : o0 + W], ones_t[:], att[:, NB : 2 * NB],
                    start=False, stop=True,
                )
            for rr in range(2):
                att = atts[rr]
                o0 = rr * W
                xt0 = xt_sb[:, 2 * (r + rr), :]
                xt1 = xt_sb[:, 2 * (r + rr) + 1, :]
                nc.tensor.matmul(
                    out_ps[:, o0 : o0 + NB], xt0, att[:, 0:NB],
                    start=True, stop=True,
                )
                nc.tensor.matmul(
                    out_ps[:, o0 + 120 : o0 + NB], xt1, att[:, NB : NB + 16],
                    start=False, stop=True, skip_group_check=True,
                )
                nc.tensor.matmul(
                    out_ps[:, o0 + NB : o0 + W], xt1, att[:, NB + 16 : 2 * NB],
                    start=True, stop=True,
                )
            rden = sbp.tile([C, 2 * W], F32, tag="rd")
            nc.vector.reciprocal_approx_fast(out=rden[:], in_=den_ps[:])
            ostage = sbp.tile([C, 2 * W], F16, tag="ost")
            nc.vector.tensor_tensor(ostage[:], out_ps[:], rden[:], MULT)
            nc.gpsimd.dma_start(out_ext[:, r : r + 2, :], ostage[:])

        # software-pipelined PE program: scores of pair p+1 are emitted
        # before den/value of pair p so PE never idles on exp(p)
        NP = ROWS // 2
        for pr in range(4):
            g_block(2 * pr)
        pend = scores_block(0)
        for p in range(NP):
            if p == 3:
                for pr in range(4):
                    g_block(8 + 2 * pr)
            nxt = scores_block(2 * (p + 1)) if p + 1 < NP else None
            denval_block(2 * p, pend)
            pend = nxt

    nc.compile()
    return nc


def host_consts(Wq, bq, Wk, bk):
    sc = np.float32(np.sqrt(np.float32(C)))
    Wq = Wq.astype(np.float64)
    Wk = Wk.astype(np.float64)
    bq = bq.astype(np.float64)
    bk = bk.astype(np.float64)
    M = (Wq.T @ Wk) / sc
    v = (Wk.T @ bq) / sc
    u = (Wq.T @ bk) / sc
    c0 = float(bq @ bk) / sc

    mt = np.ascontiguousarray(M.T).astype(np.float16)      # lhsT for g = M x
    vt = v.reshape(C, 1).astype(np.float16)
    ut = u.reshape(C, 1).astype(np.float32)
    ident = np.eye(C, dtype=np.float16)
    ones = np.ones((C, C), dtype=np.float16)

    # additive band mask in the banded block layout:
    # chunk0 cols 0..NB-1 (query w=col, key p),
    # chunk1 cols NB..2NB-1 (query w=120+(col-NB), key 128+p)
    mask = np.full((C, 2 * NB), -30000.0, dtype=np.float64)
    for pp in range(C):
        for col in range(NB):
            if abs(col - pp) <= R:
                mask[pp, col] = 0.0
            if abs((W - NB + col) - (128 + pp)) <= R:
                mask[pp, NB + col] = 0.0
    mask = mask.astype(np.float16)

    wgrid = np.arange(W)
    oob_row = np.maximum(0, R - wgrid) + np.maximum(0, wgrid - (W - 1 - R))
    oob16 = np.tile(
        (oob_row * np.exp(-c0 - S0) / C).astype(np.float16), (C, 1)
    )
    return mt, vt, ut, ident, ones, mask, oob16


def core_inputs(feature, position, Wq, bq, Wk, bk):
    mt, vt, ut, ident, ones, mask, oob16 = host_consts(Wq, bq, Wk, bk)
    in_maps = []
    for i in range(NCORES):
        b = i // CORES_PER_B
        h0 = (i % CORES_PER_B) * ROWS
        in_maps.append(
            {
                "feature": np.ascontiguousarray(
                    feature[b, :, h0 : h0 + ROWS, :], dtype=np.float32
                ),
                "position": np.ascontiguousarray(
                    position[b, :, h0 : h0 + ROWS, :], dtype=np.float32
                ),
                "mt": mt,
                "ones": ones,
                "ident": ident,
                "vt": vt,
                "ut": ut,
                "maskT": mask,
                "oob16": oob16,
            }
        )
    return in_maps


def kernel(feature, position, Wq, bq, Wk, bk):
    feature = np.asarray(feature, dtype=np.float32)
    position = np.asarray(position, dtype=np.float32)
    Wq = np.asarray(Wq, dtype=np.float32)
    bq = np.asarray(bq, dtype=np.float32)
    Wk = np.asarray(Wk, dtype=np.float32)
    bk = np.asarray(bk, dtype=np.float32)
    in_maps = core_inputs(feature, position, Wq, bq, Wk, bk)
    nc = build_nc()
    res = run_bass_kernel_spmd(nc, in_maps, list(range(NCORES)))
    out = np.empty((B, C, H, W), dtype=np.float32)
    for i in range(NCORES):
        b = i // CORES_PER_B
        h0 = (i % CORES_PER_B) * ROWS
        out[b, :, h0 : h0 + ROWS, :] = res.results[i]["out"].astype(np.float32)
    return out


# revision 17
# speedup vs baseline: 1.0752x; 1.0173x over previous
"""Sliding-window (radius-8, K=17) single-head attention along W — v4.

Math (see host_consts): S[w',w] = g(w')·x(w) + alpha(w') (+terms that cancel
in softmax), g = M x + u with M = Wq^T Wk/sc; alpha rides the ACT exp bias;
the global constant c0 and fp16-range shift s0 are baked into the zero-pad
denominator term. Banded 136-col score/den/value matmuls in fp16, with an
additive -30000 band mask accumulated into the score PSUM by a PE matmul
(exp of masked entries underflows to 0, so den/value read exp output as-is).

Engine assignment per row of [C=128, W=256]:
  Sync   : input f/p chunk DMAs (HWDGE), x^T X-bar DMA-transposes, out DMAs
  Scalar : const DMAs (parallel queue), exp with alpha bias, alpha evict
  GpSimd : x = f + p (f32+f32 -> fp16)
  PE     : g (N=512/pair), alpha (N=1), mask+scores, den, value
  DVE    : g eviction (+u -> fp16), reciprocal of den, final normalize mul
"""

import numpy as np
from contextlib import ExitStack

import concourse.bacc as bacc
import concourse.mybir as mybir
import concourse.tile as tile
from concourse.bass_utils import run_bass_kernel_spmd

B, C, H, W = 2, 128, 64, 256
R = 8
NCORES = 8
ROWS = B * H // NCORES        # 16 (b, h) rows per core
CORES_PER_B = NCORES // B     # 4
F32 = mybir.dt.float32
F16 = mybir.dt.float16
EXP = mybir.ActivationFunctionType.Exp
MULT = mybir.AluOpType.mult
S0 = 7.0                      # constant score shift (softmax-invariant)
NB = 136                      # banded block width (128 + R)
CHUNK_ROWS = [2, 2, 4, 4, 4]  # input/x chunking (small first chunks)


def build_nc():
    nc = bacc.Bacc(trn_type="TRN2")
    f_ext = nc.dram_tensor("feature", [C, ROWS, W], F32, kind="ExternalInput")
    p_ext = nc.dram_tensor("position", [C, ROWS, W], F32, kind="ExternalInput")
    mt_ext = nc.dram_tensor("mt", [C, C], F16, kind="ExternalInput")
    ones_ext = nc.dram_tensor("ones", [C, C], F16, kind="ExternalInput")
    id_ext = nc.dram_tensor("ident", [C, C], F16, kind="ExternalInput")
    v_ext = nc.dram_tensor("vt", [C, 1], F16, kind="ExternalInput")
    u_ext = nc.dram_tensor("ut", [C, 1], F32, kind="ExternalInput")
    mask_ext = nc.dram_tensor("maskT", [C, 2 * NB], F16, kind="ExternalInput")
    oob_ext = nc.dram_tensor("oob16", [C, W], F16, kind="ExternalInput")
    out_ext = nc.dram_tensor("out", [C, ROWS, W], F16, kind="ExternalOutput")

    with tile.TileContext(nc) as tc, ExitStack() as ctx:
        const = ctx.enter_context(tc.tile_pool(name="const", bufs=1))
        xgp = ctx.enter_context(tc.tile_pool(name="xg", bufs=1))
        inp = ctx.enter_context(tc.tile_pool(name="inp", bufs=5))

        # input chunks on the Sync HWDGE queue, all issued up front
        x_sb = xgp.tile([C, ROWS * W], F16, tag="x")
        fts, pts = [], []
        r0s, szs = [], []
        rr0 = 0
        for nr in CHUNK_ROWS:
            r0s.append(rr0)
            szs.append(nr * W)
            ft = inp.tile([C, 4 * W], F32, tag="ft")
            nc.sync.dma_start(ft[:, : nr * W], f_ext[:, rr0 : rr0 + nr, :])
            pt = inp.tile([C, 4 * W], F32, tag="pt")
            nc.sync.dma_start(pt[:, : nr * W], p_ext[:, rr0 : rr0 + nr, :])
            fts.append(ft)
            pts.append(pt)
            rr0 += nr

        # consts on the Scalar HWDGE queue (parallel to the input loads)
        def cload(shape, dt, ext, tag):
            t = const.tile(shape, dt, tag=tag, name=tag)
            nc.scalar.dma_start(t[:], ext[:])
            return t

        mt_t = cload([C, C], F16, mt_ext, "mt")
        ones_t = cload([C, C], F16, ones_ext, "ones")
        ident = cload([C, C], F16, id_ext, "id")
        v_t = cload([C, 1], F16, v_ext, "v")
        u_t = cload([C, 1], F32, u_ext, "u")
        mask_t = cload([C, 2 * NB], F16, mask_ext, "mask")
        oob_t = cload([C, W], F16, oob_ext, "oob")

        # x = f + p as fp16, split into 2-row ops so each pair's x (and its
        # transpose) is ready as early as possible; the first two adds run
        # on DVE (idle during the ramp), the rest on gpsimd
        nadd = 0
        for i, nr in enumerate(CHUNK_ROWS):
            for j in range(0, nr, 2):
                sl = slice((r0s[i] + j) * W, (r0s[i] + j + 2) * W)
                fsl = slice(j * W, (j + 2) * W)
                eng = nc.vector if nadd < 2 else nc.gpsimd
                eng.tensor_add(x_sb[:, sl], fts[i][:, fsl], pts[i][:, fsl])
                nadd += 1

        # x^T per chunk via X-bar DMA transpose, issued as soon as each
        # x chunk exists: xt[p, j, c] = x[c, j*128 + p]
        xt_sb = xgp.tile([C, 2 * ROWS, 128], F16, tag="xt")
        for i, nr in enumerate(CHUNK_ROWS):
            sl = slice(r0s[i] * W, r0s[i] * W + szs[i])
            nc.sync.dma_start_transpose(
                xt_sb[:, 2 * r0s[i] : 2 * (r0s[i] + nr), :], x_sb[:, sl]
            )

        # touch Exp once so the ACT table loads during the input-DMA ramp
        warm = const.tile([C, 1], F32, tag="warm")
        nc.scalar.activation(warm[:], u_t[:], EXP)
        negs0 = const.tile([C, 1], F32, tag="negs0")
        nc.vector.memset(negs0[:], -S0)

        g_sb = xgp.tile([C, ROWS * W], F16, tag="g")

        attp = ctx.enter_context(tc.tile_pool(name="att", bufs=4))
        sbp = ctx.enter_context(tc.tile_pool(name="sb", bufs=2))
        ps_s = ctx.enter_context(tc.tile_pool(name="ps_s", bufs=3, space="PSUM"))
        ps_g = ctx.enter_context(tc.tile_pool(name="ps_g", bufs=2, space="PSUM"))
        ps_dn = ctx.enter_context(tc.tile_pool(name="ps_dn", bufs=2, space="PSUM"))
        ps_o = ctx.enter_context(tc.tile_pool(name="ps_o", bufs=1, space="PSUM"))

        # g = M x (+u at eviction): one N=512 matmul per row-pair, in two
        # 4-pair bursts (M^T stationary loaded once per burst)
        def g_block(r):
            xsl = slice(r * W, (r + 2) * W)
            g_ps = ps_g.tile([C, 2 * W], F32, tag="g")
            nc.tensor.matmul(g_ps[:], mt_t[:], x_sb[:, xsl], start=True, stop=True)
            nc.vector.tensor_scalar_add(g_sb[:, xsl], g_ps[:], u_t[:])

        # scores for one pair: alpha MMs, band mask, banded score MMs,
        # then exp with alpha bias (ACT) -> att tiles
        def scores_block(r):
            s_tiles = []
            # alpha MMs first (x chunks as stationaries), both rows
            for rr in range(2):
                x_r = x_sb[:, (r + rr) * W : (r + rr + 1) * W]
                s_ps = ps_s.tile([C, 2 * NB + 2], F32, tag="s")
                s_tiles.append(s_ps)
                nc.tensor.matmul(
                    s_ps[:, 2 * NB : 2 * NB + 1], x_r[:, 0:128], v_t[:],
                    start=True, stop=True,
                )
                nc.tensor.matmul(
                    s_ps[:, 2 * NB + 1 : 2 * NB + 2], x_r[:, 128:256], v_t[:],
                    start=True, stop=True,
                )
            # band mask for both rows with ident loaded once
            for rr in range(2):
                nc.tensor.matmul(
                    s_tiles[rr][:, 0 : 2 * NB], ident[:], mask_t[:],
                    start=True, stop=False,
                )
            atts = []
            for rr in range(2):
                x_r = x_sb[:, (r + rr) * W : (r + rr + 1) * W]
                g_r = g_sb[:, (r + rr) * W : (r + rr + 1) * W]
                s_ps = s_tiles[rr]
                nc.tensor.matmul(
                    s_ps[:, 0:NB], g_r[:, 0:128], x_r[:, 0:NB],
                    start=False, stop=False,
                )
                nc.tensor.matmul(
                    s_ps[:, NB : 2 * NB], g_r[:, 128:256], x_r[:, W - NB : W],
                    start=False, stop=True,
                )
                al_sb = sbp.tile([C, 2], F32, tag="al")
                nc.scalar.add(al_sb[:], s_ps[:, 2 * NB : 2 * NB + 2], negs0[:])
                att = attp.tile([C, 2 * NB], F16, tag="att")
                nc.scalar.activation(
                    att[:, 0:NB], s_ps[:, 0:NB], EXP, bias=al_sb[:, 0:1]
                )
                nc.scalar.activation(
                    att[:, NB : 2 * NB], s_ps[:, NB : 2 * NB], EXP,
                    bias=al_sb[:, 1:2],
                )
                atts.append(att)
            return atts

        def denval_block(r, atts):
            den_ps = ps_dn.tile([C, 2 * W], F32, tag="dn")
            out_ps = ps_o.tile([C, 2 * W], F32, tag="out")
            for rr in range(2):
                att = atts[rr]
                o0 = rr * W
                nc.tensor.matmul(
                    den_ps[:, o0 : o0 + W], ones_t[:], oob_t[:],
                    start=True, stop=False,
                )
                nc.tensor.matmul(
                    den_ps[:, o0 : o0 + NB], ones_t[:], att[:, 0:NB],
                    start=False, stop=False,
                )
                nc.tensor.matmul(
                    den_ps[:, o0 + W - NB : o0 + W], ones_t[:], att[:, NB : 2 * NB],
                    start=False, stop=True,
                )
            for rr in range(2):
                att = atts[rr]
                o0 = rr * W
                xt0 = xt_sb[:, 2 * (r + rr), :]
                xt1 = xt_sb[:, 2 * (r + rr) + 1, :]
                nc.tensor.matmul(
                    out_ps[:, o0 : o0 + NB], xt0, att[:, 0:NB],
                    start=True, stop=True,
                )
                nc.tensor.matmul(
                    out_ps[:, o0 + 120 : o0 + NB], xt1, att[:, NB : NB + 16],
                    start=False, stop=True, skip_group_check=True,
                )
                nc.tensor.matmul(
                    out_ps[:, o0 + NB : o0 + W], xt1, att[:, NB + 16 : 2 * NB],
                    start=True, stop=True,
                )
            rden = sbp.tile([C, 2 * W], F32, tag="rd")
            nc.vector.reciprocal_approx_fast(out=rden[:], in_=den_ps[:])
            ostage = sbp.tile([C, 2 * W], F16, tag="ost")
            nc.vector.tensor_tensor(ostage[:], out_ps[:], rden[:], MULT)
            nc.gpsimd.dma_start(out_ext[:, r : r + 2, :], ostage[:])

        # software-pipelined PE program: scores of pair p+1 are emitted
        # before den/value of pair p so PE never idles on exp(p)
        NP = ROWS // 2
        for pr in range(4):
            g_block(2 * pr)
        pend = scores_block(0)
        for p in range(NP):
            if p == 1:
                for pr in range(4):
                    g_block(8 + 2 * pr)
            nxt = scores_block(2 * (p + 1)) if p + 1 < NP else None
            denval_block(2 * p, pend)
            pend = nxt

    nc.compile()
    return nc


def host_consts(Wq, bq, Wk, bk):
    sc = np.float32(np.sqrt(np.float32(C)))
    Wq = Wq.astype(np.float64)
    Wk = Wk.astype(np.float64)
    bq = bq.astype(np.float64)
    bk = bk.astype(np.float64)
    M = (Wq.T @ Wk) / sc
    v = (Wk.T @ bq) / sc
    u = (Wq.T @ bk) / sc
    c0 = float(bq @ bk) / sc

    mt = np.ascontiguousarray(M.T).astype(np.float16)      # lhsT for g = M x
    vt = v.reshape(C, 1).astype(np.float16)
    ut = u.reshape(C, 1).astype(np.float32)
    ident = np.eye(C, dtype=np.float16)
    ones = np.ones((C, C), dtype=np.float16)

    # additive band mask in the banded block layout:
    # chunk0 cols 0..NB-1 (query w=col, key p),
    # chunk1 cols NB..2NB-1 (query w=120+(col-NB), key 128+p)
    mask = np.full((C, 2 * NB), -30000.0, dtype=np.float64)
    for pp in range(C):
        for col in range(NB):
            if abs(col - pp) <= R:
                mask[pp, col] = 0.0
            if abs((W - NB + col) - (128 + pp)) <= R:
                mask[pp, NB + col] = 0.0
    mask = mask.astype(np.float16)

    wgrid = np.arange(W)
    oob_row = np.maximum(0, R - wgrid) + np.maximum(0, wgrid - (W - 1 - R))
    oob16 = np.tile(
        (oob_row * np.exp(-c0 - S0) / C).astype(np.float16), (C, 1)
    )
    return mt, vt, ut, ident, ones, mask, oob16


def core_inputs(feature, position, Wq, bq, Wk, bk):
    mt, vt, ut, ident, ones, mask, oob16 = host_consts(Wq, bq, Wk, bk)
    in_maps = []
    for i in range(NCORES):
        b = i // CORES_PER_B
        h0 = (i % CORES_PER_B) * ROWS
        in_maps.append(
            {
                "feature": np.ascontiguousarray(
                    feature[b, :, h0 : h0 + ROWS, :], dtype=np.float32
                ),
                "position": np.ascontiguousarray(
                    position[b, :, h0 : h0 + ROWS, :], dtype=np.float32
                ),
                "mt": mt,
                "ones": ones,
                "ident": ident,
                "vt": vt,
                "ut": ut,
                "maskT": mask,
                "oob16": oob16,
            }
        )
    return in_maps


def kernel(feature, position, Wq, bq, Wk, bk):
    feature = np.asarray(feature, dtype=np.float32)
    position = np.asarray(position, dtype=np.float32)
    Wq = np.asarray(Wq, dtype=np.float32)
    bq = np.asarray(bq, dtype=np.float32)
    Wk = np.asarray(Wk, dtype=np.float32)
    bk = np.asarray(bk, dtype=np.float32)
    in_maps = core_inputs(feature, position, Wq, bq, Wk, bk)
    nc = build_nc()
    res = run_bass_kernel_spmd(nc, in_maps, list(range(NCORES)))
    out = np.empty((B, C, H, W), dtype=np.float32)
    for i in range(NCORES):
        b = i // CORES_PER_B
        h0 = (i % CORES_PER_B) * ROWS
        out[b, :, h0 : h0 + ROWS, :] = res.results[i]["out"].astype(np.float32)
    return out


# revision 20
# speedup vs baseline: 1.1140x; 1.0360x over previous
"""Sliding-window (radius-8, K=17) single-head attention along W.

Full problem: feature/position [2, 128, 64, 256] f32; 1x1 convs Wq/Wk (+bias)
produce q/k; scores over a 17-wide window along W; softmax (zero-padded
windows contribute exp(0)=1 to the denominator); output is the attn-weighted
sum of windows of x = feature + position.

Sharding: data-parallel over (B, H) — the 128 (b, h) rows are independent;
each of the 8 cores gets 16 rows, two per iteration. Per row
(x_row = [C=128, W=256]):
  q = (Wq/sqrt(C)) x + bq/sqrt(C);  k = Wk x + bk        (PE matmuls + bias)
  S^T[w', w] = k^T q + bandmask^T   computed TRANSPOSED (keys on partitions)
      so that exp(S^T) lands in SBUF already in the layout the output matmul
      needs — no attention transposes. The mask is pre-written into PSUM by a
      PE copy-matmul (ident.T @ maskT); score matmuls accumulate on top.
  attU = exp(S^T) bf16                                   (unnormalized)
  den[w] (broadcast to all partitions) = ones128.T @ attU, accumulated on top
      of ident.T @ oob_bc (the zero-padded out-of-range counts, exp(0)=1
      each); normalization happens at the end: out = (x @ attU) * recip(den).
  out_u = x^T.T @ attU  (PE transposes of x, then accumulating matmuls)

Precision: the score path (x, Wq/Wk, q, k, S) runs in fp32r so exp() sees
near-fp32 scores; the value path (attU, x^T, output matmuls) runs in bf16
(fast weight load + 1 cyc/row). Scores accumulate in fp32 PSUM; softmax
skips max-subtraction (scores are O(10), well inside exp/fp32 range; the
unnormalized attU and den stay in range too). Measured ~53 us/core on HW
with absmax-relative error ~2.7e-3 vs the fp32 reference.
"""

import numpy as np
from contextlib import ExitStack

import concourse.bacc as bacc
import concourse.mybir as mybir
import concourse.tile as tile
from concourse.bass_utils import run_bass_kernel_spmd

B, C, H, W = 2, 128, 64, 256
R = 8
NCORES = 8
ROWS = B * H // NCORES        # 16 (b, h) rows per core
CORES_PER_B = NCORES // B     # 4
F32 = mybir.dt.float32
F32R = mybir.dt.float32r
BF = mybir.dt.bfloat16
EXP = mybir.ActivationFunctionType.Exp
NEG = -1e9
RL = 4                        # rows per input DMA


def build_nc():
    nc = bacc.Bacc(trn_type="TRN2")
    f_ext = nc.dram_tensor("feature", [C, ROWS, W], F32, kind="ExternalInput")
    p_ext = nc.dram_tensor("position", [C, ROWS, W], F32, kind="ExternalInput")
    wq_ext = nc.dram_tensor("wqt", [C, C], F32R, kind="ExternalInput")
    wk_ext = nc.dram_tensor("wkt", [C, C], F32R, kind="ExternalInput")
    id_ext = nc.dram_tensor("ident", [C, C], BF, kind="ExternalInput")
    ones_ext = nc.dram_tensor("ones", [C, C], BF, kind="ExternalInput")
    bq_ext = nc.dram_tensor("bqv", [C, 1], F32, kind="ExternalInput")
    bk_ext = nc.dram_tensor("bkv", [C, 1], F32, kind="ExternalInput")
    mask_ext = nc.dram_tensor("maskT", [C, 2 * W], BF, kind="ExternalInput")
    oob_ext = nc.dram_tensor("oob_bc", [C, 2 * W], BF, kind="ExternalInput")
    out_ext = nc.dram_tensor("out", [C, ROWS, W], F32, kind="ExternalOutput")

    with tile.TileContext(nc) as tc, ExitStack() as ctx:
        const = ctx.enter_context(tc.tile_pool(name="const", bufs=1))
        inp = ctx.enter_context(tc.tile_pool(name="inp", bufs=4))

        # all input tiles load up front on the Sync HWDGE queue (no WAR
        # waits with bufs=4); constants go on the Scalar HWDGE queue so
        # they never sit behind the bulk input transfers
        fts, pts = [], []
        for c4 in range(ROWS // RL):
            ftc = inp.tile([C, RL, W], F32, tag="ft", name="ftc")
            nc.sync.dma_start(ftc[:], f_ext[:, c4 * RL : (c4 + 1) * RL, :])
            ptc = inp.tile([C, RL, W], F32, tag="pt", name="ptc")
            nc.sync.dma_start(ptc[:], p_ext[:, c4 * RL : (c4 + 1) * RL, :])
            fts.append(ftc)
            pts.append(ptc)

        def cload(shape, dt, ext, tag):
            t = const.tile(shape, dt, tag=tag, name=tag)
            nc.scalar.dma_start(t[:], ext[:])
            return t

        wq_t = cload([C, C], F32R, wq_ext, "wq")
        wk_t = cload([C, C], F32R, wk_ext, "wk")
        ident = cload([C, C], BF, id_ext, "id")
        ones_t = cload([C, C], BF, ones_ext, "ones")
        bq_t = cload([C, 1], F32, bq_ext, "bq")
        bk_t = cload([C, 1], F32, bk_ext, "bk")
        mask_t = cload([C, 2 * W], BF, mask_ext, "mask")
        oob_t = cload([C, 2 * W], BF, oob_ext, "oob")

        # touch Exp once so the ACT table loads during the input-DMA ramp
        warm = const.tile([C, 1], F32, tag="warm")
        nc.scalar.activation(warm[:], bq_t[:], EXP)

        xp = ctx.enter_context(tc.tile_pool(name="x", bufs=4))
        qkp = ctx.enter_context(tc.tile_pool(name="qk", bufs=4))
        attp = ctx.enter_context(tc.tile_pool(name="att", bufs=4))
        sbT = ctx.enter_context(tc.tile_pool(name="sbT", bufs=4))
        rdp = ctx.enter_context(tc.tile_pool(name="rd", bufs=4))
        psqk = ctx.enter_context(tc.tile_pool(name="psqk", bufs=2, space="PSUM"))
        pss = ctx.enter_context(tc.tile_pool(name="pss", bufs=3, space="PSUM"))
        psxt = ctx.enter_context(tc.tile_pool(name="psxt", bufs=1, space="PSUM"))
        pso = ctx.enter_context(tc.tile_pool(name="pso", bufs=2, space="PSUM"))

        for it in range(ROWS // 2):
            r = 2 * it
            ft = fts[r // RL]
            pt = pts[r // RL]
            j = r % RL

            # x per row: f32r for the conv/score path (gpsimd), bf16 copy
            # for the value path (DVE)
            xt32 = xp.tile([C, 2, W], F32R, tag="x32")
            nc.gpsimd.tensor_add(xt32[:, 0], ft[:, j, :], pt[:, j, :])
            nc.gpsimd.tensor_add(xt32[:, 1], ft[:, j + 1, :], pt[:, j + 1, :])
            xt = xp.tile([C, 2, W], BF)
            nc.vector.tensor_copy(xt[:], xt32[:])

            # q|k per row: [C, 512] fp32 PSUM (1 bank each)
            # qk_sb layout: q rows at [0 : 2W], k rows at [2W : 4W]
            qk_sb = qkp.tile([C, 4 * W], F32R)
            for rr in range(2):
                qk_ps = psqk.tile([C, 2 * W], F32, tag="qk")
                nc.tensor.matmul(
                    qk_ps[:, 0:W], wq_t[:], xt32[:, rr], start=True, stop=True
                )
                nc.tensor.matmul(
                    qk_ps[:, W : 2 * W], wk_t[:], xt32[:, rr], start=True, stop=True
                )
                nc.scalar.add(
                    qk_sb[:, rr * W : (rr + 1) * W], qk_ps[:, 0:W], bq_t[:]
                )
                nc.vector.tensor_scalar_add(
                    qk_sb[:, 2 * W + rr * W : 2 * W + (rr + 1) * W],
                    qk_ps[:, W : 2 * W],
                    bk_t[:],
                )

            # attU^T per row, straight to SBUF: att[:, r*512:(r+1)*512] is
            # [keys-chunk (2x128 partitions) | queries 0:256 free] per row.
            att = attp.tile([C, 4 * W], BF)
            for rr in range(2):
                q0 = rr * W
                k0 = 2 * W + rr * W
                s_ps = pss.tile([C, 2 * W], F32, tag="s")
                nc.tensor.matmul(s_ps[:], ident[:], mask_t[:], start=True, stop=False)
                nc.tensor.matmul(
                    s_ps[:, 0:W],
                    qk_sb[:, k0 : k0 + 128],
                    qk_sb[:, q0 : q0 + 2 * 128],
                    start=False, stop=False,
                )
                nc.tensor.matmul(
                    s_ps[:, W : 2 * W],
                    qk_sb[:, k0 + 128 : k0 + W],
                    qk_sb[:, q0 : q0 + 2 * 128],
                    start=False, stop=True,
                )
                nc.scalar.activation(
                    att[:, rr * 2 * W : (rr + 1) * 2 * W], s_ps[:], EXP
                )

            # denominators, broadcast across partitions by the ones matmul;
            # oob counts pre-accumulated from a constant.
            den_ps = pss.tile([C, 2 * W], F32, tag="s")
            nc.tensor.matmul(den_ps[:], ident[:], oob_t[:], start=True, stop=False)
            for rr in range(2):
                a0 = rr * 2 * W
                nc.tensor.matmul(
                    den_ps[:, rr * W : (rr + 1) * W],
                    ones_t[:],
                    att[:, a0 : a0 + W],
                    start=False, stop=False,
                )
                nc.tensor.matmul(
                    den_ps[:, rr * W : (rr + 1) * W],
                    ones_t[:],
                    att[:, a0 + W : a0 + 2 * W],
                    start=False, stop=(rr == 1),
                )
            rden = rdp.tile([C, 2 * W], F32)
            nc.vector.reciprocal_approx_fast(out=rden[:], in_=den_ps[:])

            # x^T chunks for the output matmul
            xt_ps = psxt.tile([C, 2 * W], BF, tag="xt")
            for rr in range(2):
                nc.tensor.transpose(
                    xt_ps[:, rr * W : rr * W + 128], xt[:, rr, 0:128], ident[:]
                )
                nc.tensor.transpose(
                    xt_ps[:, rr * W + 128 : (rr + 1) * W], xt[:, rr, 128:256], ident[:]
                )
            xT = sbT.tile([C, 2 * W], BF, tag="xT")
            nc.vector.tensor_copy(xT[:], xt_ps[:])

            o_ps = pso.tile([C, 2 * W], F32, tag="out")
            for rr in range(2):
                os_ = o_ps[:, rr * W : (rr + 1) * W]
                a0 = rr * 2 * W
                nc.tensor.matmul(
                    os_,
                    xT[:, rr * W : rr * W + 128],
                    att[:, a0 : a0 + W],
                    start=True, stop=False,
                )
                nc.tensor.matmul(
                    os_,
                    xT[:, rr * W + 128 : (rr + 1) * W],
                    att[:, a0 + W : a0 + 2 * W],
                    start=False, stop=True,
                )
            o_sb = sbT.tile([C, 2 * W], F32, tag="osb")
            nc.vector.tensor_mul(o_sb[:], o_ps[:], rden[:])
            nc.sync.dma_start(out_ext[:, r : r + 2, :], o_sb[:])

    nc.compile()
    return nc


def host_consts(Wq, bq, Wk, bk):
    import ml_dtypes

    sc = 1.0 / np.sqrt(np.float32(C))
    wqt = np.ascontiguousarray(Wq.astype(np.float32).T * sc)
    bqv = np.ascontiguousarray((bq.astype(np.float32) * sc).reshape(C, 1))
    wkt = np.ascontiguousarray(Wk.astype(np.float32).T)
    bkv = np.ascontiguousarray(bk.astype(np.float32).reshape(C, 1))

    ident = np.eye(C, dtype=np.float32).astype(ml_dtypes.bfloat16)
    ones = np.ones((C, C), dtype=np.float32).astype(ml_dtypes.bfloat16)

    # maskT[p, c*W + w] for key chunk c: key w' = c*128 + p, query w
    wgrid = np.arange(W)
    maskT = np.full((C, 2 * W), NEG, dtype=np.float32)
    for cchunk in range(2):
        for p in range(C):
            wk_ = cchunk * 128 + p
            lo, hi = max(0, wk_ - R), min(W, wk_ + R + 1)
            maskT[p, cchunk * W + lo : cchunk * W + hi] = 0.0
    maskT = maskT.astype(ml_dtypes.bfloat16)

    # oob count per query w, same row repeated on all partitions, two rows
    oob_row = np.maximum(0, R - wgrid) + np.maximum(0, wgrid - (W - 1 - R))
    oob_bc = np.tile(oob_row.astype(np.float32), (C, 2)).astype(ml_dtypes.bfloat16)
    return wqt, bqv, wkt, bkv, maskT, oob_bc, ident, ones


def core_inputs(feature, position, Wq, bq, Wk, bk):
    wqt, bqv, wkt, bkv, maskT, oob_bc, ident, ones = host_consts(Wq, bq, Wk, bk)
    in_maps = []
    for i in range(NCORES):
        b = i // CORES_PER_B
        h0 = (i % CORES_PER_B) * ROWS
        in_maps.append(
            {
                "feature": np.ascontiguousarray(
                    feature[b, :, h0 : h0 + ROWS, :], dtype=np.float32
                ),
                "position": np.ascontiguousarray(
                    position[b, :, h0 : h0 + ROWS, :], dtype=np.float32
                ),
                "wqt": wqt,
                "ident": ident,
                "ones": ones,
                "wkt": wkt,
                "bqv": bqv,
                "bkv": bkv,
                "maskT": maskT,
                "oob_bc": oob_bc,
            }
        )
    return in_maps


def kernel(feature, position, Wq, bq, Wk, bk):
    feature = np.asarray(feature, dtype=np.float32)
    position = np.asarray(position, dtype=np.float32)
    Wq = np.asarray(Wq, dtype=np.float32)
    bq = np.asarray(bq, dtype=np.float32)
    Wk = np.asarray(Wk, dtype=np.float32)
    bk = np.asarray(bk, dtype=np.float32)
    in_maps = core_inputs(feature, position, Wq, bq, Wk, bk)
    nc = build_nc()
    res = run_bass_kernel_spmd(nc, in_maps, list(range(NCORES)))
    out = np.empty((B, C, H, W), dtype=np.float32)
    for i in range(NCORES):
        b = i // CORES_PER_B
        h0 = (i % CORES_PER_B) * ROWS
        out[b, :, h0 : h0 + ROWS, :] = res.results[i]["out"]
    return out

